# revision 9
# baseline (speedup 1.0000x reference)
"""Trainium2 Bass kernel for nn_DN (topk_masking): cosine top-1 winner-take-all.

Math (reference):
    xf    = l2norm(x.reshape(B, -1))            # [B, X]
    w_xy  = l2norm_rows(x2y_w)                  # [Y, X]
    y_pre = (xf @ w_xy.T) * (y_age >= 1)        # [B, Y]
    win   = argmax(y_pre, axis=1)               # [B]
    out   = l2norm_rows(y2z_w)[:, win].T        # [B, Z]

Key observations used here:
  * ||x_b|| > 0 scales a whole row of y_pre -> does not affect argmax; x is
    never normalized on device.
  * The per-y scale (2^10 * mask_y / ||w_y||) is folded into the fp8 weights
    on the host: masked rows become exact zeros (matching the reference's
    masked-score-0 semantics, which matter when every active score of a row
    is negative), active rows are pre-normalized, so the matmul result is
    directly the gated, normalized score (up to the shared 2^10 * ||x_b||
    factor, which cancels in the argmax).
  * out row b is just column win[b] of the row-normalized y2z_w -> an
    indirect-DMA gather from a host-prenormalized bf16 table, not a matmul.

Sharding: Y (32768) split across 8 cores (4096 each). Each core computes
scores for its Y-slice with fp8 x fp8 DoubleRow TensorE matmuls (2x the
bf16 rate): each instruction contracts a 256-k pair into a [64 b, 256 y]
PSUM tile (DoubleRow requires dst partition 0, so scores live on 64-row
tiles; batch runs as 8 tiles of 64 b, in two passes of 4 over the fully
resident fp8 weights). xt/wt are host-pre-arranged into the exact SBUF
image so every DMA descriptor is a long contiguous per-partition run.
Per-(b, 256-y-group) top-8 via the DVE max8 unit, local per-core winner
resolution, a single tiny AllGather (per-core winner value+index, 4KB),
PE-transpose of the gathered rows, global winner resolve (max value, ties
-> lowest y, matching jnp.argmax), then an indirect-DMA gather of the
winning pre-normalized bf16 y2z rows straight to the output.

Scoring error (fp8 rounding of both x and the normalized weights) is
bounded empirically well under DELTA in x-normalized units; the kernel
also outputs all top-8 candidate values/indices per (core, b, group). The
host re-checks every row whose top-2 margin is within 2*DELTA, rescoring
the few in-band candidates in fp64 and patching rows where fp8 flipped
the argmax. The 8th-candidate tail guard makes this airtight: if a
group's weakest reported candidate is still in band, the row is fully
rescored. All bulk math -- the 137 GFLOP of scoring matmuls, top-8,
winner resolution, output gather -- happens on device.
"""

from dataclasses import dataclass

import numpy as np
import ml_dtypes

import concourse.bass as bass
import concourse.mybir as mybir
import concourse.tile as tile
from concourse import bacc
from concourse.bass_utils import run_bass_kernel_spmd


def _dedup_ldweights(m):
    """Drop InstLdweights whose stationary image is already in the PE array.

    The DoubleRow matmul streams its 256 moving columns in ~53ns but the
    LDWEIGHTS that precedes it costs ~118ns and does not hide, so the kernel
    emits kp rounds of 4 matmuls sharing one stationary and this pass
    removes the 3 redundant reloads (walrus's own ldw-opt rejects IR with
    explicit InstLdweights). A removed ld's semaphore waits/updates move to
    the matmul that followed it; engine in-order execution keeps the stall
    point equivalent.
    """
    removed = 0
    for fn in m.functions:
        for blk in fn.blocks:
            out = []
            last_sig = None
            pend_w, pend_u = [], []
            for inst in blk.instructions:
                tn = type(inst).__name__
                if getattr(inst, "engine", None) != mybir.EngineType.PE:
                    out.append(inst)
                    continue
                if tn == "InstLdweights":
                    ap = inst.ins[0]
                    sig = (str(ap.ap), ap.offset, str(ap.dtype), ap.memref,
                           str(inst.perf_mode), bool(inst.is_transpose),
                           str(inst.tile_position), str(inst.tile_size))
                    if sig == last_sig:
                        si = inst.sync_info
                        if si is not None:
                            pend_w.extend(si.on_wait)
                            pend_u.extend(si.on_update)
                        removed += 1
                        continue
                    last_sig = sig
                elif tn == "InstMatmult":
                    if inst.is_transpose:
                        last_sig = None   # self-loading, clobbers the plane
                elif tn in ("InstEventSemaphore", "InstNop", "InstDrain"):
                    pass                  # no effect on the weight plane
                else:
                    last_sig = None       # unknown PE op: be conservative
                if pend_w or pend_u:
                    si = inst.sync_info
                    ow = list(si.on_wait) if si is not None else []
                    ou = list(si.on_update) if si is not None else []
                    inst.sync_info = mybir.SyncInfo(
                        on_wait=ow + pend_w, on_update=ou + pend_u)
                    pend_w, pend_u = [], []
                out.append(inst)
            assert not (pend_w or pend_u), "dangling ldweights syncs"
            blk.instructions = out
    return removed

P = 128
BF16 = mybir.dt.bfloat16
FP8 = mybir.dt.float8e4
F32 = mybir.dt.float32
U32 = mybir.dt.uint32

WSCALE = 1024.0          # power-of-2 scale keeping normalized w out of fp8 subnormals


@dataclass(frozen=True)
class Geom:
    B: int = 512          # batch
    X: int = 4096         # input features
    Y: int = 32768        # y neurons (sharded)
    Z: int = 1000         # output classes
    NC: int = 8           # cores
    GW: int = 256         # y-group width (one [64, GW] f32 PSUM bank tile)

    @property
    def JT(self): return self.B // 64         # 64-b tiles
    @property
    def KP(self): return self.X // 256        # k-tile pairs (DoubleRow)
    @property
    def YL(self): return self.Y // self.NC    # y per core
    @property
    def G(self): return self.YL // self.GW    # y groups per core
    @property
    def CAND(self): return self.JT * self.G * 8
    @property
    def ZPB(self):                             # padded Z (256B bf16 rows)
        return ((self.Z * 2 + 255) // 256) * 256 // 2


FULL = Geom()

# Margin (in x-normalized score units) below which the host re-checks a row.
# Scoring uses fp8e4m3 for both x and the pre-normalized weights; the
# resulting score error on this distribution is ~2.6e-3 max at X=4096
# (empirically ~4.4e-4 std); 5e-3 is ~2x that observed max. The
# 8th-candidate band guard makes the re-check airtight under this bound.
DELTA = 5e-3

TRACE = False          # test harness sets True (needs NTFF hook installed)
TRACE_KWARGS = {}
LAST_RESULTS = None    # BassKernelResults of the last run (for profiling)


# --------------------------------------------------------------------------
# device kernel
# --------------------------------------------------------------------------

def build_nc(g: Geom = FULL) -> bacc.Bacc:
    nc = bacc.Bacc("TRN2", target_bir_lowering=False, debug=False,
                   num_devices=g.NC)

    G8 = g.G * 8
    XCOLS = g.KP * 2 * g.B            # xt sbuf cols per partition
    GCH = g.KP * 2 * g.GW             # wt sbuf cols per group chunk
    CCN = 2 * g.JT * 64               # AllGather floats per core

    # xt/wt are pre-arranged on the host into the exact SBUF image so every
    # DMA descriptor is one long contiguous run per partition
    xt_d = nc.dram_tensor("xt", [P, XCOLS], FP8, kind="ExternalInput")
    wt_d = nc.dram_tensor("wt", [g.G, P, GCH], FP8, kind="ExternalInput")
    base_d = nc.dram_tensor("base", [64, g.CAND], F32, kind="ExternalInput")
    eye_d = nc.dram_tensor("eye8", [8, 8], F32, kind="ExternalInput")
    w2t_d = nc.dram_tensor("w2t", [g.Y, g.ZPB], BF16, kind="ExternalInput")

    out_d = nc.dram_tensor("out", [g.B, g.Z], BF16, kind="ExternalOutput")
    candv_d = nc.dram_tensor("candv", [64, g.CAND], F32, kind="ExternalOutput")
    candi_d = nc.dram_tensor("candi", [64, g.CAND], F32, kind="ExternalOutput")

    with tile.TileContext(nc) as tc:
        with (
            tc.tile_pool(name="big_p", bufs=1) as big_p,
            tc.tile_pool(name="cand_p", bufs=1) as cand_p,
            tc.tile_pool(name="misc_p", bufs=1) as misc_p,
            tc.tile_pool(name="post_p", bufs=1) as post_p,
            tc.tile_pool(name="s_ps", bufs=8, space="PSUM") as s_ps,
            tc.tile_pool(name="dram_p", bufs=1, space="DRAM") as dram_p,
        ):
            # ---- resident input DMAs (first matmul needs wt g0 + xt c0) ---
            wt_sb = big_p.tile([P, g.G * GCH], FP8, tag="wt")

            def wt_chunk(gi, parts):
                cs = GCH // parts
                for s in range(parts):
                    nc.sync.dma_start(
                        out=wt_sb[:, gi * GCH + s * cs:gi * GCH + (s + 1) * cs],
                        in_=wt_d.ap()[gi, :, s * cs:(s + 1) * cs])

            xt_sb = big_p.tile([P, XCOLS], FP8, tag="xt")
            nxc = 4
            xcs = XCOLS // nxc

            def xt_chunk(s):
                nc.sync.dma_start(out=xt_sb[:, s * xcs:(s + 1) * xcs],
                                  in_=xt_d.ap()[:, s * xcs:(s + 1) * xcs])

            # first group-quad interleaved kp-quarter-wise with xt so the PE
            # can start consuming block (j=0, gb=0) as chunks land
            GB0 = min(4, g.G)
            qcs = GCH // 4
            xt_chunk(0)
            for q in range(4):
                for gi in range(GB0):
                    nc.sync.dma_start(
                        out=wt_sb[:, gi * GCH + q * qcs:
                                  gi * GCH + (q + 1) * qcs],
                        in_=wt_d.ap()[gi, :, q * qcs:(q + 1) * qcs])
                if q < 3:
                    xt_chunk(q + 1)
            for gi in range(GB0, g.G):
                wt_chunk(gi, 2)

            # ---- small resident setup -------------------------------------
            mi = misc_p.tile([64, G8 + 8 + 2 * g.JT], F32, tag="mi")
            o = [0]

            def _col(n):
                c = o[0]; o[0] += n
                return mi[:, c:c + n]
            bigG8 = _col(G8)
            big8 = _col(8)
            winv = _col(g.JT)
            wini = _col(g.JT)
            nc.gpsimd.memset(bigG8, 1e30)
            nc.gpsimd.memset(big8, 1e30)
            base_sb = misc_p.tile([64, g.CAND], F32, tag="base")
            nc.sync.dma_start(out=base_sb[:], in_=base_d.ap())
            eye8 = misc_p.tile([8, 8], F32, tag="eye8")
            nc.sync.dma_start(out=eye8[:], in_=eye_d.ap())
            candv_sb = cand_p.tile([64, g.CAND], F32, tag="candv")
            candiu_sb = cand_p.tile([64, g.CAND], U32, tag="candiu")

            ccin = dram_p.tile([CCN], F32)
            ccout = dram_p.tile([g.NC, CCN], F32, addr_space="Shared")

            # ---- main loop: group-quad blocks x 64-b tiles, DoubleRow fp8 -
            # Inner order (kp, gg): the 4 matmuls of a kp round share one
            # stationary xt tile, so walrus's ldw-opt collapses their
            # LDWEIGHTS to one (the load otherwise dominates the 53ns
            # DoubleRow stream). gb-major block order keeps each group-quad
            # resident across all 8 b-tiles so the wt DMA only has to beat
            # the first block, not every pass.
            xt4 = xt_sb[:].rearrange("p (kp i b) -> p kp i b", kp=g.KP, i=2)
            wt4 = wt_sb[:].rearrange("p (g kp i n) -> p g kp i n",
                                     g=g.G, kp=g.KP, i=2)
            NGB = min(4, g.G)
            for gb in range(0, g.G, NGB):
                for j in range(g.JT):
                    sps = [s_ps.tile([64, g.GW], F32, tag="s",
                                     name=f"s{gb}_{j}_{t}")
                           for t in range(NGB)]
                    for kp in range(g.KP):
                        for t in range(NGB):
                            nc.tensor.matmul(
                                sps[t][:, :],
                                xt4[:, kp, :, j * 64:(j + 1) * 64],
                                wt4[:, gb + t, kp, :, :],
                                start=(kp == 0), stop=(kp == g.KP - 1),
                                perf_mode=mybir.MatmulPerfMode.DoubleRow)

                    # per-(b, group) top8 straight off PSUM
                    for t in range(NGB):
                        c0 = j * G8 + (gb + t) * 8
                        nc.vector.max(candv_sb[:, c0:c0 + 8], sps[t][:])
                        nc.vector.max_index(candiu_sb[:, c0:c0 + 8],
                                            candv_sb[:, c0:c0 + 8], sps[t][:])

            # ---- stage 2: winner resolution + output gather ---------------
            # globalize candidate indices
            candi_sb = cand_p.tile([64, g.CAND], F32, tag="candi")
            nc.vector.tensor_copy(candi_sb[:], candiu_sb[:])
            nc.vector.tensor_add(candi_sb[:], candi_sb[:], base_sb[:])

            # per-core winner per b: max value, ties -> lowest global y
            for j in range(g.JT):
                cv = candv_sb[:, j * G8:(j + 1) * G8]
                ci = candi_sb[:, j * G8:(j + 1) * G8]
                nc.vector.tensor_reduce(winv[:, j:j + 1], cv,
                                        axis=mybir.AxisListType.X,
                                        op=mybir.AluOpType.max)
                eq = cand_p.tile([64, G8], mybir.dt.int32, tag="eq")
                nc.vector.tensor_scalar(eq[:], cv, winv[:, j:j + 1], None,
                                        op0=mybir.AluOpType.is_equal)
                sel = cand_p.tile([64, G8], F32, tag="sel")
                nc.vector.select(sel[:], eq[:], ci, bigG8)
                nc.vector.tensor_reduce(wini[:, j:j + 1], sel[:],
                                        axis=mybir.AxisListType.X,
                                        op=mybir.AluOpType.min)

            # AllGather the per-core winners. ccin is packed partition-major
            # so each DMA is one contiguous run per partition.
            jt_p = g.JT * 64
            nc.sync.dma_start(
                out=ccin[0:jt_p].rearrange("(p t) -> p t", p=64), in_=winv)
            nc.sync.dma_start(
                out=ccin[jt_p:2 * jt_p].rearrange("(p t) -> p t", p=64),
                in_=wini)
            nc.gpsimd.collective_compute(
                "AllGather", mybir.AluOpType.bypass,
                replica_groups=[list(range(g.NC))],
                ins=[ccin[:].opt()], outs=[ccout[:].opt()])

            # candidate dumps for the host-side margin check (off the
            # critical path)
            nc.sync.dma_start(out=candv_d.ap(), in_=candv_sb[:])
            nc.sync.dma_start(out=candi_d.ap(), in_=candi_sb[:])

            # read ccout as 8 contiguous rows, then PE-transpose [8, 64]
            # chunks into PSUM so the resolve sees [b-part, core-free]
            ccsb = post_p.tile([8, CCN], F32, tag="ccsb")
            nc.sync.dma_start(out=ccsb[:], in_=ccout[:, :])
            ccv = ccsb[0:8, 0:jt_p].rearrange("c (p t) -> c t p", t=g.JT)
            cci = ccsb[0:8, jt_p:2 * jt_p].rearrange("c (p t) -> c t p",
                                                     t=g.JT)
            tr = s_ps.tile([64, 16 * g.JT], F32, tag="s")
            for j in range(g.JT):
                nc.tensor.transpose(tr[:, j * 16:j * 16 + 8],
                                    ccv[:, j, :], eye8[0:8, :])
                nc.tensor.transpose(tr[:, j * 16 + 8:j * 16 + 16],
                                    cci[:, j, :], eye8[0:8, :])

            # global winner per b + gather of pre-normalized bf16 y2z rows
            pa = post_p.tile([64, (2 + g.NC) * g.JT], F32, tag="pa")
            po = [0]

            def _pcol(n):
                c = po[0]; po[0] += n
                return pa[:, c:c + n]
            v1_all = _pcol(g.JT)
            sel8_all = _pcol(g.NC * g.JT)
            wif_all = _pcol(g.JT)
            ia = post_p.tile([64, 2 * g.NC + g.JT], U32, tag="ia")
            wiu_all = ia[:, 2 * g.NC:2 * g.NC + g.JT]
            for j in range(g.JT):
                av = tr[:, j * 16:j * 16 + 8]
                ai = tr[:, j * 16 + 8:j * 16 + 16]
                v1 = v1_all[:, j:j + 1]
                nc.vector.tensor_reduce(v1, av, axis=mybir.AxisListType.X,
                                        op=mybir.AluOpType.max)
                eq8 = ia[:, (j % 2) * g.NC:(j % 2) * g.NC + g.NC]
                nc.vector.tensor_scalar(eq8, av, v1, None,
                                        op0=mybir.AluOpType.is_equal)
                sel8 = sel8_all[:, j * g.NC:(j + 1) * g.NC]
                nc.vector.select(sel8, eq8, ai, big8[:, 0:g.NC])
                wif = wif_all[:, j:j + 1]
                nc.vector.tensor_reduce(wif, sel8,
                                        axis=mybir.AxisListType.X,
                                        op=mybir.AluOpType.min)
                nc.vector.tensor_copy(wiu_all[:, j:j + 1], wif)

            grows = [post_p.tile([64, g.ZPB], BF16, name=f"grow{j}",
                                 tag=f"grow{j}")
                     for j in range(g.JT)]
            for j in range(g.JT):
                nc.gpsimd.indirect_dma_start(
                    out=grows[j][:], out_offset=None,
                    in_=w2t_d.ap(),
                    in_offset=bass.IndirectOffsetOnAxis(
                        ap=wiu_all[:, j:j + 1], axis=0))
                nc.sync.dma_start(
                    out=out_d.ap()[j * 64:(j + 1) * 64, :],
                    in_=grows[j][:, 0:g.Z])

    _dedup_ldweights(nc.m)
    nc.compile()
    return nc


# --------------------------------------------------------------------------
# host side
# --------------------------------------------------------------------------

def prep_inputs(g: Geom, x, x2y_w, y2z_w, y_age):
    """Shard + lay out the full inputs for the 8 cores."""
    fp8 = ml_dtypes.float8_e4m3
    bf16 = ml_dtypes.bfloat16
    KP = g.KP
    G8 = g.G * 8

    xf = np.ascontiguousarray(x.reshape(g.B, g.X))
    # xt image: xt[p, kp*2B + i*B + b] = x[b, (kp*2+i)*128 + p]
    xt = np.ascontiguousarray(
        xf.astype(fp8).T.reshape(KP, 2, P, g.B).transpose(2, 0, 1, 3)
        .reshape(P, KP * 2 * g.B))

    # fold (2^10 * mask / ||w_y||) into the weights, then fp8-quantize
    wn = np.linalg.norm(x2y_w, axis=1)
    scale = (WSCALE / np.maximum(wn, 1e-12)) * (y_age[0] >= 1)
    wq = (x2y_w * scale[:, None].astype(np.float32)).astype(fp8)   # [Y, X]

    # pre-normalized bf16 output table: w2t[y, z] = y2z_w[z, y]/||y2z_w[z,:]||
    n2 = np.linalg.norm(y2z_w.astype(np.float64), axis=1)
    w2t = np.zeros((g.Y, g.ZPB), bf16)
    w2t[:, :g.Z] = (y2z_w / n2[:, None]).T.astype(bf16)

    eye8 = np.eye(8, dtype=np.float32)
    in_maps = []
    for c in range(g.NC):
        ys = slice(c * g.YL, (c + 1) * g.YL)
        wslc = wq[ys, :]                                  # [YL, X] fp8
        # wt group chunks: [G, P, KP*2*GW], chunk gi is the SBUF image of
        # one 256-y group (contiguous per partition):
        #   wt[gi, p, kp*2GW + i*GW + n] = w[gi*GW + n, (kp*2+i)*128 + p]
        wt = np.ascontiguousarray(
            wslc.T.reshape(KP, 2, P, g.G, g.GW)
            .transpose(3, 2, 0, 1, 4).reshape(g.G, P, KP * 2 * g.GW))
        cols = np.arange(g.CAND)
        base_row = (c * g.YL + g.GW * ((cols % G8) // 8)).astype(np.float32)
        base = np.broadcast_to(base_row, (64, g.CAND)).copy()
        in_maps.append({"xt": xt, "wt": wt, "base": base, "w2t": w2t,
                        "eye8": eye8})
    return in_maps


def postprocess(g: Geom, results, x, x2y_w, y2z_w, y_age):
    """Margin check + fp64 rescore of close rows; patch flipped winners."""
    out = np.array(results[0]["out"], dtype=np.float32, copy=True)
    G8 = g.G * 8
    # candidate values/indices -> [B, NC * G8]; b = j*64 + p
    V = np.empty((g.B, g.NC * G8), np.float32)
    I = np.empty((g.B, g.NC * G8), np.float32)
    for c in range(g.NC):
        cv = np.asarray(results[c]["candv"])   # [64, CAND]
        ci = np.asarray(results[c]["candi"])
        for j in range(g.JT):
            V[j * 64:(j + 1) * 64, c * G8:(c + 1) * G8] = \
                cv[:, j * G8:(j + 1) * G8]
            I[j * 64:(j + 1) * 64, c * G8:(c + 1) * G8] = \
                ci[:, j * G8:(j + 1) * G8]

    xf = x.reshape(g.B, g.X).astype(np.float64)
    xn = np.linalg.norm(xf, axis=1)
    mask = (y_age[0] >= 1)
    n2 = np.linalg.norm(y2z_w.astype(np.float64), axis=1)

    def exact_c(b, ys):
        ys = np.asarray(ys, dtype=np.int64)
        W = x2y_w[ys, :].astype(np.float64)
        c = (W @ xf[b]) / np.linalg.norm(W, axis=1) / xn[b]
        return np.where(mask[ys], c, 0.0)

    n_flagged = n_patched = 0
    full_rows = []
    for b in range(g.B):
        vb, ib = V[b], I[b]
        vmax = vb.max()
        band = 2.0 * DELTA * xn[b] * WSCALE
        in_band = vb >= vmax - band
        if int(in_band.sum()) <= 1:
            continue
        n_flagged += 1
        # guard: if any group's 8th (weakest reported) candidate is still in
        # band, candidates may be missing -> full exact rescore of the row
        tails = vb.reshape(-1, 8)[:, 7]
        if np.any(tails >= vmax - band):
            full_rows.append(b)
        else:
            dev_w = int(ib[vb == vmax].min())
            ys = np.unique(ib[in_band].astype(np.int64))
            ce = exact_c(b, ys)
            cbest = ce.max()
            w_true = int(ys[ce == cbest].min())
            if w_true != dev_w:
                n_patched += 1
                out[b, :] = (y2z_w[:, w_true].astype(np.float64)
                             / n2).astype(np.float32)
    if full_rows:
        W = x2y_w.astype(np.float64)
        wnorm = np.linalg.norm(W, axis=1)
        call = (xf[full_rows] @ W.T) / wnorm[None, :] \
            / xn[full_rows][:, None]
        call = np.where(mask[None, :], call, 0.0)
        for r, b in enumerate(full_rows):
            vb, ib = V[b], I[b]
            dev_w = int(ib[vb == vb.max()].min())
            cbest = call[r].max()
            w_true = int(np.nonzero(call[r] == cbest)[0].min())
            if w_true != dev_w:
                n_patched += 1
                out[b, :] = (y2z_w[:, w_true].astype(np.float64)
                             / n2).astype(np.float32)
    postprocess.stats = {"flagged": n_flagged, "patched": n_patched,
                         "full_rescore": len(full_rows)}
    return out


_BUILT = {}


def _get_nc(g: Geom):
    if g not in _BUILT:
        _BUILT[g] = build_nc(g)
    return _BUILT[g]


def kernel(**inputs) -> np.ndarray:
    global LAST_RESULTS
    g = FULL
    x = np.asarray(inputs["x"], dtype=np.float32)
    x2y_w = np.asarray(inputs["x2y_w"], dtype=np.float32)
    y2z_w = np.asarray(inputs["y2z_w"], dtype=np.float32)
    y_age = np.asarray(inputs["y_age"])

    nc = _get_nc(g)
    in_maps = prep_inputs(g, x, x2y_w, y2z_w, y_age)
    res = run_bass_kernel_spmd(nc, in_maps, list(range(g.NC)),
                               trace=TRACE, **TRACE_KWARGS)
    LAST_RESULTS = res
    return postprocess(g, res.results, x, x2y_w, y2z_w, y_age)


# revision 17
# speedup vs baseline: 1.0438x; 1.0438x over previous
"""Trainium2 Bass kernel for nn_DN (topk_masking): cosine top-1 winner-take-all.

Math (reference):
    xf    = l2norm(x.reshape(B, -1))            # [B, X]
    w_xy  = l2norm_rows(x2y_w)                  # [Y, X]
    y_pre = (xf @ w_xy.T) * (y_age >= 1)        # [B, Y]
    win   = argmax(y_pre, axis=1)               # [B]
    out   = l2norm_rows(y2z_w)[:, win].T        # [B, Z]

Key observations used here:
  * ||x_b|| > 0 scales a whole row of y_pre -> does not affect argmax; x is
    never normalized on device.
  * The per-y scale (2^10 * mask_y / ||w_y||) is folded into the fp8 weights
    on the host: masked rows become exact zeros (matching the reference's
    masked-score-0 semantics, which matter when every active score of a row
    is negative), active rows are pre-normalized, so the matmul result is
    directly the gated, normalized score (up to the shared 2^10 * ||x_b||
    factor, which cancels in the argmax).
  * out row b is just column win[b] of the row-normalized y2z_w -> an
    indirect-DMA gather from a host-prenormalized bf16 table, not a matmul.

Sharding: Y (32768) split across 8 cores (4096 each). Each core computes
scores for its Y-slice with fp8 x fp8 DoubleRow TensorE matmuls (2x the
bf16 rate): each instruction contracts a 256-k pair into a [64 b, 256 y]
PSUM tile (DoubleRow requires dst partition 0, so scores live on 64-row
tiles; batch runs as 8 tiles of 64 b, in two passes of 4 over the fully
resident fp8 weights). xt/wt are host-pre-arranged into the exact SBUF
image so every DMA descriptor is a long contiguous per-partition run.
Per-(b, 256-y-group) top-8 via the DVE max8 unit, local per-core winner
resolution, a single tiny AllGather (per-core winner value+index, 4KB),
PE-transpose of the gathered rows, global winner resolve (max value, ties
-> lowest y, matching jnp.argmax), then an indirect-DMA gather of the
winning pre-normalized bf16 y2z rows straight to the output.

Scoring error (fp8 rounding of both x and the normalized weights) is
bounded empirically well under DELTA in x-normalized units; the kernel
also outputs all top-8 candidate values/indices per (core, b, group). The
host re-checks every row whose top-2 margin is within 2*DELTA, rescoring
the few in-band candidates in fp64 and patching rows where fp8 flipped
the argmax. The 8th-candidate tail guard makes this airtight: if a
group's weakest reported candidate is still in band, the row is fully
rescored. All bulk math -- the 137 GFLOP of scoring matmuls, top-8,
winner resolution, output gather -- happens on device.
"""

from dataclasses import dataclass

import numpy as np
import ml_dtypes

import concourse.bass as bass
import concourse.mybir as mybir
import concourse.tile as tile
from concourse import bacc
from concourse.bass_utils import run_bass_kernel_spmd


def _dedup_ldweights(m):
    """Drop InstLdweights whose stationary image is already in the PE array.

    The DoubleRow matmul streams its 256 moving columns in ~53ns but the
    LDWEIGHTS that precedes it costs ~118ns and does not hide, so the kernel
    emits kp rounds of 4 matmuls sharing one stationary and this pass
    removes the 3 redundant reloads (walrus's own ldw-opt rejects IR with
    explicit InstLdweights). A removed ld's semaphore waits/updates move to
    the matmul that followed it; engine in-order execution keeps the stall
    point equivalent.
    """
    removed = 0
    for fn in m.functions:
        for blk in fn.blocks:
            out = []
            last_sig = None
            pend_w, pend_u = [], []
            for inst in blk.instructions:
                tn = type(inst).__name__
                if getattr(inst, "engine", None) != mybir.EngineType.PE:
                    out.append(inst)
                    continue
                if tn == "InstLdweights":
                    ap = inst.ins[0]
                    sig = (str(ap.ap), ap.offset, str(ap.dtype), ap.memref,
                           str(inst.perf_mode), bool(inst.is_transpose),
                           str(inst.tile_position), str(inst.tile_size))
                    if sig == last_sig:
                        si = inst.sync_info
                        if si is not None:
                            pend_w.extend(si.on_wait)
                            pend_u.extend(si.on_update)
                        removed += 1
                        continue
                    last_sig = sig
                elif tn == "InstMatmult":
                    if inst.is_transpose:
                        last_sig = None   # self-loading, clobbers the plane
                elif tn in ("InstEventSemaphore", "InstNop", "InstDrain"):
                    pass                  # no effect on the weight plane
                else:
                    last_sig = None       # unknown PE op: be conservative
                if pend_w or pend_u:
                    si = inst.sync_info
                    ow = list(si.on_wait) if si is not None else []
                    ou = list(si.on_update) if si is not None else []
                    inst.sync_info = mybir.SyncInfo(
                        on_wait=ow + pend_w, on_update=ou + pend_u)
                    pend_w, pend_u = [], []
                out.append(inst)
            assert not (pend_w or pend_u), "dangling ldweights syncs"
            blk.instructions = out
    return removed

P = 128
BF16 = mybir.dt.bfloat16
FP8 = mybir.dt.float8e4
F32 = mybir.dt.float32
U32 = mybir.dt.uint32

WSCALE = 1024.0          # power-of-2 scale keeping normalized w out of fp8 subnormals


@dataclass(frozen=True)
class Geom:
    B: int = 512          # batch
    X: int = 4096         # input features
    Y: int = 32768        # y neurons (sharded)
    Z: int = 1000         # output classes
    NC: int = 8           # cores
    GW: int = 256         # y-group width (one [64, GW] f32 PSUM bank tile)

    @property
    def JT(self): return self.B // 64         # 64-b tiles
    @property
    def KP(self): return self.X // 256        # k-tile pairs (DoubleRow)
    @property
    def YL(self): return self.Y // self.NC    # y per core
    @property
    def G(self): return self.YL // self.GW    # y groups per core
    @property
    def CAND(self): return self.JT * self.G * 8
    @property
    def ZPB(self):                             # padded Z (256B bf16 rows)
        return ((self.Z * 2 + 255) // 256) * 256 // 2


FULL = Geom()

# Margin (in x-normalized score units) below which the host re-checks a row.
# Scoring uses fp8e4m3 for both x and the pre-normalized weights; the
# resulting score error on this distribution is ~2.6e-3 max at X=4096
# (empirically ~4.4e-4 std); 5e-3 is ~2x that observed max. The
# 8th-candidate band guard makes the re-check airtight under this bound.
DELTA = 5e-3

TRACE = False          # test harness sets True (needs NTFF hook installed)
TRACE_KWARGS = {}
LAST_RESULTS = None    # BassKernelResults of the last run (for profiling)


# --------------------------------------------------------------------------
# device kernel
# --------------------------------------------------------------------------

def build_nc(g: Geom = FULL) -> bacc.Bacc:
    nc = bacc.Bacc("TRN2", target_bir_lowering=False, debug=False,
                   num_devices=g.NC)

    G8 = g.G * 8
    XCOLS = g.KP * 2 * g.B            # xt sbuf cols per partition
    GCH = g.KP * 2 * g.GW             # wt sbuf cols per group chunk
    CCN = g.JT * 64                   # AllReduce floats (one packed per b)

    # xt/wt are pre-arranged on the host into the exact SBUF image so every
    # DMA descriptor is one long contiguous run per partition
    xt_d = nc.dram_tensor("xt", [P, XCOLS], FP8, kind="ExternalInput")
    wt_d = nc.dram_tensor("wt", [g.G, P, GCH], FP8, kind="ExternalInput")
    base_d = nc.dram_tensor("base", [64, g.CAND], F32, kind="ExternalInput")
    w2t_d = nc.dram_tensor("w2t", [g.Y, g.ZPB], BF16, kind="ExternalInput")

    out_d = nc.dram_tensor("out", [g.B, g.Z], BF16, kind="ExternalOutput")
    candv_d = nc.dram_tensor("candv", [64, g.CAND], F32, kind="ExternalOutput")
    candi_d = nc.dram_tensor("candi", [64, g.CAND], F32, kind="ExternalOutput")

    with tile.TileContext(nc) as tc:
        with (
            tc.tile_pool(name="big_p", bufs=1) as big_p,
            tc.tile_pool(name="cand_p", bufs=1) as cand_p,
            tc.tile_pool(name="misc_p", bufs=1) as misc_p,
            tc.tile_pool(name="post_p", bufs=1) as post_p,
            tc.tile_pool(name="s_ps", bufs=8, space="PSUM") as s_ps,
            tc.tile_pool(name="dram_p", bufs=1, space="DRAM") as dram_p,
        ):
            # ---- resident input DMAs (first matmul needs wt g0 + xt c0) ---
            wt_sb = big_p.tile([P, g.G * GCH], FP8, tag="wt")

            def wt_chunk(gi, parts):
                cs = GCH // parts
                for s in range(parts):
                    nc.sync.dma_start(
                        out=wt_sb[:, gi * GCH + s * cs:gi * GCH + (s + 1) * cs],
                        in_=wt_d.ap()[gi, :, s * cs:(s + 1) * cs])

            xt_sb = big_p.tile([P, XCOLS], FP8, tag="xt")
            nxc = 4
            xcs = XCOLS // nxc

            def xt_chunk(s):
                nc.sync.dma_start(out=xt_sb[:, s * xcs:(s + 1) * xcs],
                                  in_=xt_d.ap()[:, s * xcs:(s + 1) * xcs])

            # first group-quad interleaved kp-quarter-wise with xt so the PE
            # can start consuming block (j=0, gb=0) as chunks land
            GB0 = min(4, g.G)
            qcs = GCH // 4
            xt_chunk(0)
            for q in range(4):
                for gi in range(GB0):
                    nc.sync.dma_start(
                        out=wt_sb[:, gi * GCH + q * qcs:
                                  gi * GCH + (q + 1) * qcs],
                        in_=wt_d.ap()[gi, :, q * qcs:(q + 1) * qcs])
                if q < 3:
                    xt_chunk(q + 1)
            for gi in range(GB0, g.G):
                wt_chunk(gi, 2)

            # ---- small resident setup -------------------------------------
            mi = misc_p.tile([64, G8 + 4 * g.JT], F32, tag="mi")
            o = [0]

            def _col(n):
                c = o[0]; o[0] += n
                return mi[:, c:c + n]
            bigG8 = _col(G8)
            winv = _col(g.JT)
            wini = _col(g.JT)
            kq = _col(g.JT)
            pkt = _col(g.JT)
            nc.gpsimd.memset(bigG8, 1e30)
            base_sb = misc_p.tile([64, g.CAND], F32, tag="base")
            nc.sync.dma_start(out=base_sb[:], in_=base_d.ap())
            candv_sb = cand_p.tile([64, g.CAND], F32, tag="candv")
            candiu_sb = cand_p.tile([64, g.CAND], U32, tag="candiu")

            ccin = dram_p.tile([CCN], F32)
            ccout = dram_p.tile([CCN], F32, addr_space="Shared")

            # ---- main loop: group-quad blocks x 64-b tiles, DoubleRow fp8 -
            # Inner order (kp, gg): the 4 matmuls of a kp round share one
            # stationary xt tile, so walrus's ldw-opt collapses their
            # LDWEIGHTS to one (the load otherwise dominates the 53ns
            # DoubleRow stream). gb-major block order keeps each group-quad
            # resident across all 8 b-tiles so the wt DMA only has to beat
            # the first block, not every pass.
            xt4 = xt_sb[:].rearrange("p (kp i b) -> p kp i b", kp=g.KP, i=2)
            wt4 = wt_sb[:].rearrange("p (g kp i n) -> p g kp i n",
                                     g=g.G, kp=g.KP, i=2)
            NGB = min(4, g.G)
            for gb in range(0, g.G, NGB):
                for j in range(g.JT):
                    sps = [s_ps.tile([64, g.GW], F32, tag="s",
                                     name=f"s{gb}_{j}_{t}")
                           for t in range(NGB)]
                    for kp in range(g.KP):
                        for t in range(NGB):
                            nc.tensor.matmul(
                                sps[t][:, :],
                                xt4[:, kp, :, j * 64:(j + 1) * 64],
                                wt4[:, gb + t, kp, :, :],
                                start=(kp == 0), stop=(kp == g.KP - 1),
                                perf_mode=mybir.MatmulPerfMode.DoubleRow)

                    # per-(b, group) top8 straight off PSUM
                    for t in range(NGB):
                        c0 = j * G8 + (gb + t) * 8
                        nc.vector.max(candv_sb[:, c0:c0 + 8], sps[t][:])
                        nc.vector.max_index(candiu_sb[:, c0:c0 + 8],
                                            candv_sb[:, c0:c0 + 8], sps[t][:])

            # ---- stage 2: winner resolution + output gather ---------------
            # globalize candidate indices
            candi_sb = cand_p.tile([64, g.CAND], F32, tag="candi")
            nc.vector.tensor_copy(candi_sb[:], candiu_sb[:])
            nc.vector.tensor_add(candi_sb[:], candi_sb[:], base_sb[:])

            # batched local winner per (b, j): max value, ties -> lowest y
            cv3 = candv_sb[:].rearrange("p (j c) -> p j c", j=g.JT)
            ci3 = candi_sb[:].rearrange("p (j c) -> p j c", j=g.JT)
            nc.vector.tensor_reduce(winv, cv3, axis=mybir.AxisListType.X,
                                    op=mybir.AluOpType.max)
            eqm = cand_p.tile([64, g.CAND], mybir.dt.int32, tag="eqm")
            eqm3 = eqm[:].rearrange("p (j c) -> p j c", j=g.JT)
            wb3 = winv.unsqueeze(2).broadcast_to([64, g.JT, G8])
            nc.vector.tensor_tensor(eqm3, cv3, wb3,
                                    op=mybir.AluOpType.is_equal)
            selm = cand_p.tile([64, g.CAND], F32, tag="selm")
            selm3 = selm[:].rearrange("p (j c) -> p j c", j=g.JT)
            bb3 = bigG8.unsqueeze(1).broadcast_to([64, g.JT, G8])
            nc.vector.select(selm3, eqm3, ci3, bb3)
            nc.vector.tensor_reduce(wini, selm3, axis=mybir.AxisListType.X,
                                    op=mybir.AluOpType.min)

            # pack (value, index) into one sortable float per (b):
            #   packed = (256 - round(clamp(v)/32)) * 2^15 + y
            # AllReduce(min) then resolves max value, ties -> lowest y.
            # All quantities stay exact in fp32 (integers < 2^24); the host
            # replicates this arithmetic bit-for-bit and absorbs the <=32-unit
            # value quantization inside the DELTA re-check band.
            nc.vector.tensor_scalar(kq, winv, 8000.0, -8000.0,
                                    op0=mybir.AluOpType.min,
                                    op1=mybir.AluOpType.max)
            nc.vector.tensor_scalar(kq, kq, 0.03125, 12582912.0,
                                    op0=mybir.AluOpType.mult,
                                    op1=mybir.AluOpType.add)
            nc.vector.tensor_scalar(kq, kq, 12582912.0, None,
                                    op0=mybir.AluOpType.subtract)
            nc.vector.tensor_scalar(pkt, kq, -32768.0, 8388608.0,
                                    op0=mybir.AluOpType.mult,
                                    op1=mybir.AluOpType.add)
            nc.vector.tensor_add(pkt, pkt, wini)

            nc.sync.dma_start(
                out=ccin[:].rearrange("(p j) -> p j", p=64), in_=pkt)
            nc.gpsimd.collective_compute(
                "AllReduce", mybir.AluOpType.min,
                replica_groups=[list(range(g.NC))],
                ins=[ccin[:].opt()], outs=[ccout[:].opt()])

            # candidate dumps for the host-side margin check (off the
            # critical path)
            nc.sync.dma_start(out=candv_d.ap(), in_=candv_sb[:])
            nc.sync.dma_start(out=candi_d.ap(), in_=candi_sb[:])

            # unpack the global winner index and gather the pre-normalized
            # bf16 y2z rows straight to the output
            prb = post_p.tile([64, g.JT], F32, tag="prb")
            nc.sync.dma_start(out=prb[:],
                              in_=ccout[:].rearrange("(p j) -> p j", p=64))
            pru = post_p.tile([64, g.JT], U32, tag="pru")
            nc.vector.tensor_copy(pru[:], prb[:])
            wiu = post_p.tile([64, g.JT], U32, tag="wiu")
            nc.vector.tensor_scalar(wiu[:], pru[:], 32767, None,
                                    op0=mybir.AluOpType.bitwise_and)

            grows = [post_p.tile([64, g.ZPB], BF16, name=f"grow{j}",
                                 tag=f"grow{j}")
                     for j in range(g.JT)]
            for j in range(g.JT):
                nc.gpsimd.indirect_dma_start(
                    out=grows[j][:], out_offset=None,
                    in_=w2t_d.ap(),
                    in_offset=bass.IndirectOffsetOnAxis(
                        ap=wiu[:, j:j + 1], axis=0))
                nc.sync.dma_start(
                    out=out_d.ap()[j * 64:(j + 1) * 64, :],
                    in_=grows[j][:, 0:g.Z])

    _dedup_ldweights(nc.m)
    nc.compile()
    return nc


# --------------------------------------------------------------------------
# host side
# --------------------------------------------------------------------------

def prep_inputs(g: Geom, x, x2y_w, y2z_w, y_age):
    """Shard + lay out the full inputs for the 8 cores."""
    fp8 = ml_dtypes.float8_e4m3
    bf16 = ml_dtypes.bfloat16
    KP = g.KP
    G8 = g.G * 8

    xf = np.ascontiguousarray(x.reshape(g.B, g.X))
    # xt image: xt[p, kp*2B + i*B + b] = x[b, (kp*2+i)*128 + p]
    xt = np.ascontiguousarray(
        xf.astype(fp8).T.reshape(KP, 2, P, g.B).transpose(2, 0, 1, 3)
        .reshape(P, KP * 2 * g.B))

    # fold (2^10 * mask / ||w_y||) into the weights, then fp8-quantize
    wn = np.linalg.norm(x2y_w, axis=1)
    scale = (WSCALE / np.maximum(wn, 1e-12)) * (y_age[0] >= 1)
    wq = (x2y_w * scale[:, None].astype(np.float32)).astype(fp8)   # [Y, X]

    # pre-normalized bf16 output table: w2t[y, z] = y2z_w[z, y]/||y2z_w[z,:]||
    n2 = np.linalg.norm(y2z_w.astype(np.float64), axis=1)
    w2t = np.zeros((g.Y, g.ZPB), bf16)
    w2t[:, :g.Z] = (y2z_w / n2[:, None]).T.astype(bf16)

    in_maps = []
    for c in range(g.NC):
        ys = slice(c * g.YL, (c + 1) * g.YL)
        wslc = wq[ys, :]                                  # [YL, X] fp8
        # wt group chunks: [G, P, KP*2*GW], chunk gi is the SBUF image of
        # one 256-y group (contiguous per partition):
        #   wt[gi, p, kp*2GW + i*GW + n] = w[gi*GW + n, (kp*2+i)*128 + p]
        wt = np.ascontiguousarray(
            wslc.T.reshape(KP, 2, P, g.G, g.GW)
            .transpose(3, 2, 0, 1, 4).reshape(g.G, P, KP * 2 * g.GW))
        cols = np.arange(g.CAND)
        base_row = (c * g.YL + g.GW * ((cols % G8) // 8)).astype(np.float32)
        base = np.broadcast_to(base_row, (64, g.CAND)).copy()
        in_maps.append({"xt": xt, "wt": wt, "base": base, "w2t": w2t})
    return in_maps


def postprocess(g: Geom, results, x, x2y_w, y2z_w, y_age):
    """Margin check + fp64 rescore of close rows; patch flipped winners."""
    out = np.array(results[0]["out"], dtype=np.float32, copy=True)
    G8 = g.G * 8
    # candidate values/indices -> [B, NC * G8]; b = j*64 + p
    V = np.empty((g.B, g.NC * G8), np.float32)
    I = np.empty((g.B, g.NC * G8), np.float32)
    for c in range(g.NC):
        cv = np.asarray(results[c]["candv"])   # [64, CAND]
        ci = np.asarray(results[c]["candi"])
        for j in range(g.JT):
            V[j * 64:(j + 1) * 64, c * G8:(c + 1) * G8] = \
                cv[:, j * G8:(j + 1) * G8]
            I[j * 64:(j + 1) * 64, c * G8:(c + 1) * G8] = \
                ci[:, j * G8:(j + 1) * G8]

    xf = x.reshape(g.B, g.X).astype(np.float64)
    xn = np.linalg.norm(xf, axis=1)
    mask = (y_age[0] >= 1)
    n2 = np.linalg.norm(y2z_w.astype(np.float64), axis=1)

    # replicate the device's packed AllReduce(min) winner resolution
    # bit-for-bit (fp32 clamp/scale/round, 2^15 packing) so the patch logic
    # compares against the row the device actually gathered
    Vc = V.reshape(g.B, g.NC, G8)
    Ic = I.reshape(g.B, g.NC, G8)
    vmax_c = Vc.max(axis=2)                                   # [B, NC] f32
    imin_c = np.where(Vc == vmax_c[:, :, None], Ic, np.inf).min(axis=2)
    kf = np.rint(np.clip(vmax_c, np.float32(-8000.0), np.float32(8000.0))
                 * np.float32(0.03125))
    packed = (256.0 - kf.astype(np.float64)) * 32768.0 \
        + imin_c.astype(np.float64)
    dev_rows = (packed.min(axis=1).astype(np.int64)) & 32767   # [B]

    def exact_c(b, ys):
        ys = np.asarray(ys, dtype=np.int64)
        W = x2y_w[ys, :].astype(np.float64)
        c = (W @ xf[b]) / np.linalg.norm(W, axis=1) / xn[b]
        return np.where(mask[ys], c, 0.0)

    n_flagged = n_patched = 0
    full_rows = []
    for b in range(g.B):
        vb, ib = V[b], I[b]
        vmax = vb.max()
        band = 2.0 * DELTA * xn[b] * WSCALE
        in_band = vb >= vmax - band
        if int(in_band.sum()) <= 1:
            continue
        n_flagged += 1
        # guard: if any group's 8th (weakest reported) candidate is still in
        # band, candidates may be missing -> full exact rescore of the row
        tails = vb.reshape(-1, 8)[:, 7]
        if np.any(tails >= vmax - band):
            full_rows.append(b)
        else:
            dev_w = int(dev_rows[b])
            ys = np.unique(ib[in_band].astype(np.int64))
            ce = exact_c(b, ys)
            cbest = ce.max()
            w_true = int(ys[ce == cbest].min())
            if w_true != dev_w:
                n_patched += 1
                out[b, :] = (y2z_w[:, w_true].astype(np.float64)
                             / n2).astype(np.float32)
    if full_rows:
        W = x2y_w.astype(np.float64)
        wnorm = np.linalg.norm(W, axis=1)
        call = (xf[full_rows] @ W.T) / wnorm[None, :] \
            / xn[full_rows][:, None]
        call = np.where(mask[None, :], call, 0.0)
        for r, b in enumerate(full_rows):
            dev_w = int(dev_rows[b])
            cbest = call[r].max()
            w_true = int(np.nonzero(call[r] == cbest)[0].min())
            if w_true != dev_w:
                n_patched += 1
                out[b, :] = (y2z_w[:, w_true].astype(np.float64)
                             / n2).astype(np.float32)
    postprocess.stats = {"flagged": n_flagged, "patched": n_patched,
                         "full_rescore": len(full_rows)}
    return out


_BUILT = {}


def _get_nc(g: Geom):
    if g not in _BUILT:
        _BUILT[g] = build_nc(g)
    return _BUILT[g]


def kernel(**inputs) -> np.ndarray:
    global LAST_RESULTS
    g = FULL
    x = np.asarray(inputs["x"], dtype=np.float32)
    x2y_w = np.asarray(inputs["x2y_w"], dtype=np.float32)
    y2z_w = np.asarray(inputs["y2z_w"], dtype=np.float32)
    y_age = np.asarray(inputs["y_age"])

    nc = _get_nc(g)
    in_maps = prep_inputs(g, x, x2y_w, y2z_w, y_age)
    res = run_bass_kernel_spmd(nc, in_maps, list(range(g.NC)),
                               trace=TRACE, **TRACE_KWARGS)
    LAST_RESULTS = res
    return postprocess(g, res.results, x, x2y_w, y2z_w, y_age)


# revision 23
# speedup vs baseline: 1.3475x; 1.2910x over previous
"""Trainium2 Bass kernel for nn_DN (topk_masking): cosine top-1 winner-take-all.

Math (reference):
    xf    = l2norm(x.reshape(B, -1))            # [B, X]
    w_xy  = l2norm_rows(x2y_w)                  # [Y, X]
    y_pre = (xf @ w_xy.T) * (y_age >= 1)        # [B, Y]
    win   = argmax(y_pre, axis=1)               # [B]
    out   = l2norm_rows(y2z_w)[:, win].T        # [B, Z]

Key observations used here:
  * ||x_b|| > 0 scales a whole row of y_pre -> does not affect argmax; x is
    never normalized on device.
  * The per-y scale (2^10 * mask_y / ||w_y||) is folded into the fp8 weights
    on the host: masked rows become exact zeros (matching the reference's
    masked-score-0 semantics, which matter when every active score of a row
    is negative), active rows are pre-normalized, so the matmul result is
    directly the gated, normalized score (up to the shared 2^10 * ||x_b||
    factor, which cancels in the argmax).
  * out row b is just column win[b] of the row-normalized y2z_w -> an
    indirect-DMA gather from a host-prenormalized bf16 table, not a matmul.

Sharding: Y (32768) split across 8 cores (4096 each). Each core computes
scores for its Y-slice with fp8 x fp8 DoubleRow TensorE matmuls (2x the
bf16 rate): each instruction contracts a 256-k pair into a [64 b, 256 y]
PSUM tile (DoubleRow requires dst partition 0, so scores live on 64-row
tiles; batch runs as 8 tiles of 64 b, in two passes of 4 over the fully
resident fp8 weights). xt/wt are host-pre-arranged into the exact SBUF
image so every DMA descriptor is a long contiguous per-partition run.
Per-(b, 256-y-group) top-8 via the DVE max8 unit, local per-core winner
resolution, a single tiny AllGather (per-core winner value+index, 4KB),
PE-transpose of the gathered rows, global winner resolve (max value, ties
-> lowest y, matching jnp.argmax), then an indirect-DMA gather of the
winning pre-normalized bf16 y2z rows straight to the output.

Scoring error (fp8 rounding of both x and the normalized weights) is
bounded empirically well under DELTA in x-normalized units; the kernel
also outputs all top-8 candidate values/indices per (core, b, group). The
host re-checks every row whose top-2 margin is within 2*DELTA, rescoring
the few in-band candidates in fp64 and patching rows where fp8 flipped
the argmax. The 8th-candidate tail guard makes this airtight: if a
group's weakest reported candidate is still in band, the row is fully
rescored. All bulk math -- the 137 GFLOP of scoring matmuls, top-8,
winner resolution, output gather -- happens on device.
"""

from dataclasses import dataclass

import numpy as np
import ml_dtypes

import concourse.bass as bass
import concourse.mybir as mybir
import concourse.tile as tile
from concourse import bacc
from concourse.bass_utils import run_bass_kernel_spmd


def _dedup_ldweights(m):
    """Drop InstLdweights whose stationary image is already in the PE array.

    The DoubleRow matmul streams its 256 moving columns in ~53ns but the
    LDWEIGHTS that precedes it costs ~118ns and does not hide, so the kernel
    emits kp rounds of 4 matmuls sharing one stationary and this pass
    removes the 3 redundant reloads (walrus's own ldw-opt rejects IR with
    explicit InstLdweights). A removed ld's semaphore waits/updates move to
    the matmul that followed it; engine in-order execution keeps the stall
    point equivalent.
    """
    removed = 0
    for fn in m.functions:
        for blk in fn.blocks:
            out = []
            last_sig = None
            pend_w, pend_u = [], []
            for inst in blk.instructions:
                tn = type(inst).__name__
                if getattr(inst, "engine", None) != mybir.EngineType.PE:
                    out.append(inst)
                    continue
                if tn == "InstLdweights":
                    ap = inst.ins[0]
                    sig = (str(ap.ap), ap.offset, str(ap.dtype), ap.memref,
                           str(inst.perf_mode), bool(inst.is_transpose),
                           str(inst.tile_position), str(inst.tile_size))
                    if sig == last_sig:
                        si = inst.sync_info
                        if si is not None:
                            pend_w.extend(si.on_wait)
                            pend_u.extend(si.on_update)
                        removed += 1
                        continue
                    last_sig = sig
                elif tn == "InstMatmult":
                    if inst.is_transpose:
                        last_sig = None   # self-loading, clobbers the plane
                elif tn in ("InstEventSemaphore", "InstNop", "InstDrain"):
                    pass                  # no effect on the weight plane
                else:
                    last_sig = None       # unknown PE op: be conservative
                if pend_w or pend_u:
                    si = inst.sync_info
                    ow = list(si.on_wait) if si is not None else []
                    ou = list(si.on_update) if si is not None else []
                    inst.sync_info = mybir.SyncInfo(
                        on_wait=ow + pend_w, on_update=ou + pend_u)
                    pend_w, pend_u = [], []
                out.append(inst)
            assert not (pend_w or pend_u), "dangling ldweights syncs"
            blk.instructions = out
    return removed

P = 128
BF16 = mybir.dt.bfloat16
FP8 = mybir.dt.float8e4
F32 = mybir.dt.float32
U32 = mybir.dt.uint32

WSCALE = 1024.0          # power-of-2 scale keeping normalized w out of fp8 subnormals


@dataclass(frozen=True)
class Geom:
    B: int = 512          # batch
    X: int = 4096         # input features
    Y: int = 32768        # y neurons (sharded)
    Z: int = 1000         # output classes
    NC: int = 8           # cores
    GW: int = 256         # y-group width (one [64, GW] f32 PSUM bank tile)
    YLC: int = 4096       # per-core compacted+padded y count (masked rows
                          # dropped up to one representative; see _compaction)

    @property
    def JT(self): return self.B // 64         # 64-b tiles
    @property
    def KP(self): return self.X // 256        # k-tile pairs (DoubleRow)
    @property
    def YL(self): return self.Y // self.NC    # original y per core
    @property
    def G(self): return self.YLC // self.GW   # y groups per core
    @property
    def YT(self): return self.NC * self.YLC   # compacted global y count
    @property
    def CAND(self): return self.JT * self.G * 8
    @property
    def ZPB(self):                             # padded Z (256B bf16 rows)
        return ((self.Z * 2 + 255) // 256) * 256 // 2


FULL = Geom()


def _compaction(nc_n, gw, y_age, Y):
    """Per-core y compaction: keep active rows (age>=1) plus the lowest
    masked row as the zero-score representative (masked scores are exactly 0
    in the reference and can win when every active score is negative; among
    tied zeros argmax picks the lowest index, which the representative
    preserves -- compaction keeps index order within and across cores).
    Pads (zero rows, score 0 too) sit at the tail of each core's slice, so
    they lose every min-index tie against the representative.

    Returns (YLC, gperm) where gperm maps compact global index -> original y
    (-1 for pads)."""
    ncy = Y // nc_n
    perms = []
    for c in range(nc_n):
        a = np.asarray(y_age).reshape(-1)[c * ncy:(c + 1) * ncy]
        act = np.nonzero(a >= 1)[0]
        msk = np.nonzero(a < 1)[0]
        keep = act if msk.size == 0 else np.sort(np.append(act, msk[0]))
        perms.append(keep.astype(np.int64) + c * ncy)
    # pad so G stays a multiple of the 4-group matmul block
    quant = gw * 4
    ylc = ((max(p.size for p in perms) + quant - 1) // quant) * quant
    gperm = np.full(nc_n * ylc, -1, np.int64)
    for c, p in enumerate(perms):
        gperm[c * ylc:c * ylc + p.size] = p
    return ylc, gperm

# Margin (in x-normalized score units) below which the host re-checks a row.
# Scoring uses fp8e4m3 for both x and the pre-normalized weights; the
# resulting score error on this distribution is ~2.6e-3 max at X=4096
# (empirically ~4.4e-4 std); 5e-3 is ~2x that observed max. The
# 8th-candidate band guard makes the re-check airtight under this bound.
DELTA = 5e-3

TRACE = False          # test harness sets True (needs NTFF hook installed)
TRACE_KWARGS = {}
LAST_RESULTS = None    # BassKernelResults of the last run (for profiling)


# --------------------------------------------------------------------------
# device kernel
# --------------------------------------------------------------------------

def build_nc(g: Geom = FULL) -> bacc.Bacc:
    nc = bacc.Bacc("TRN2", target_bir_lowering=False, debug=False,
                   num_devices=g.NC)

    G8 = g.G * 8
    XCOLS = g.KP * 2 * g.B            # xt sbuf cols per partition
    GCH = g.KP * 2 * g.GW             # wt sbuf cols per group chunk
    CCN = g.JT * 64                   # AllReduce floats (one packed per b)

    # xt/wt are pre-arranged on the host into the exact SBUF image so every
    # DMA descriptor is one long contiguous run per partition
    xt_d = nc.dram_tensor("xt", [P, XCOLS], FP8, kind="ExternalInput")
    wt_d = nc.dram_tensor("wt", [g.G, P, GCH], FP8, kind="ExternalInput")
    base_d = nc.dram_tensor("base", [64, g.CAND], F32, kind="ExternalInput")
    w2t_d = nc.dram_tensor("w2t", [g.YT, g.ZPB], BF16, kind="ExternalInput")

    out_d = nc.dram_tensor("out", [g.B, g.Z], BF16, kind="ExternalOutput")
    candv_d = nc.dram_tensor("candv", [64, g.CAND], F32, kind="ExternalOutput")
    candi_d = nc.dram_tensor("candi", [64, g.CAND], F32, kind="ExternalOutput")

    with tile.TileContext(nc) as tc:
        with (
            tc.tile_pool(name="big_p", bufs=1) as big_p,
            tc.tile_pool(name="cand_p", bufs=1) as cand_p,
            tc.tile_pool(name="misc_p", bufs=1) as misc_p,
            tc.tile_pool(name="post_p", bufs=1) as post_p,
            tc.tile_pool(name="s_ps", bufs=8, space="PSUM") as s_ps,
            tc.tile_pool(name="dram_p", bufs=1, space="DRAM") as dram_p,
        ):
            # ---- resident input DMAs (first matmul needs wt g0 + xt c0) ---
            wt_sb = big_p.tile([P, g.G * GCH], FP8, tag="wt")

            def wt_chunk(gi, parts):
                cs = GCH // parts
                for s in range(parts):
                    nc.sync.dma_start(
                        out=wt_sb[:, gi * GCH + s * cs:gi * GCH + (s + 1) * cs],
                        in_=wt_d.ap()[gi, :, s * cs:(s + 1) * cs])

            xt_sb = big_p.tile([P, XCOLS], FP8, tag="xt")
            nxc = 4
            xcs = XCOLS // nxc

            def xt_chunk(s):
                nc.sync.dma_start(out=xt_sb[:, s * xcs:(s + 1) * xcs],
                                  in_=xt_d.ap()[:, s * xcs:(s + 1) * xcs])

            # first group-quad interleaved kp-quarter-wise with xt so the PE
            # can start consuming block (j=0, gb=0) as chunks land
            GB0 = min(4, g.G)
            qcs = GCH // 4
            xt_chunk(0)
            for q in range(4):
                for gi in range(GB0):
                    nc.sync.dma_start(
                        out=wt_sb[:, gi * GCH + q * qcs:
                                  gi * GCH + (q + 1) * qcs],
                        in_=wt_d.ap()[gi, :, q * qcs:(q + 1) * qcs])
                if q < 3:
                    xt_chunk(q + 1)
            for gi in range(GB0, g.G):
                wt_chunk(gi, 2)

            # ---- small resident setup -------------------------------------
            mi = misc_p.tile([64, G8 + 4 * g.JT], F32, tag="mi")
            o = [0]

            def _col(n):
                c = o[0]; o[0] += n
                return mi[:, c:c + n]
            bigG8 = _col(G8)
            winv = _col(g.JT)
            wini = _col(g.JT)
            kq = _col(g.JT)
            pkt = _col(g.JT)
            nc.gpsimd.memset(bigG8, 1e30)
            base_sb = misc_p.tile([64, g.CAND], F32, tag="base")
            nc.sync.dma_start(out=base_sb[:], in_=base_d.ap())
            candv_sb = cand_p.tile([64, g.CAND], F32, tag="candv")
            candiu_sb = cand_p.tile([64, g.CAND], U32, tag="candiu")

            ccin = dram_p.tile([CCN], F32)
            ccout = dram_p.tile([CCN], F32, addr_space="Shared")

            # ---- main loop: group-quad blocks x 64-b tiles, DoubleRow fp8 -
            # Inner order (kp, gg): the 4 matmuls of a kp round share one
            # stationary xt tile, so walrus's ldw-opt collapses their
            # LDWEIGHTS to one (the load otherwise dominates the 53ns
            # DoubleRow stream). gb-major block order keeps each group-quad
            # resident across all 8 b-tiles so the wt DMA only has to beat
            # the first block, not every pass.
            xt4 = xt_sb[:].rearrange("p (kp i b) -> p kp i b", kp=g.KP, i=2)
            wt4 = wt_sb[:].rearrange("p (g kp i n) -> p g kp i n",
                                     g=g.G, kp=g.KP, i=2)
            NGB = min(4, g.G)
            for gb in range(0, g.G, NGB):
                for j in range(g.JT):
                    sps = [s_ps.tile([64, g.GW], F32, tag="s",
                                     name=f"s{gb}_{j}_{t}")
                           for t in range(NGB)]
                    for kp in range(g.KP):
                        for t in range(NGB):
                            nc.tensor.matmul(
                                sps[t][:, :],
                                xt4[:, kp, :, j * 64:(j + 1) * 64],
                                wt4[:, gb + t, kp, :, :],
                                start=(kp == 0), stop=(kp == g.KP - 1),
                                perf_mode=mybir.MatmulPerfMode.DoubleRow)

                    # per-(b, group) top8 straight off PSUM
                    for t in range(NGB):
                        c0 = j * G8 + (gb + t) * 8
                        nc.vector.max(candv_sb[:, c0:c0 + 8], sps[t][:])
                        nc.vector.max_index(candiu_sb[:, c0:c0 + 8],
                                            candv_sb[:, c0:c0 + 8], sps[t][:])

            # ---- stage 2: winner resolution + output gather ---------------
            # globalize candidate indices
            candi_sb = cand_p.tile([64, g.CAND], F32, tag="candi")
            nc.vector.tensor_copy(candi_sb[:], candiu_sb[:])
            nc.vector.tensor_add(candi_sb[:], candi_sb[:], base_sb[:])

            # batched local winner per (b, j): max value, ties -> lowest y
            cv3 = candv_sb[:].rearrange("p (j c) -> p j c", j=g.JT)
            ci3 = candi_sb[:].rearrange("p (j c) -> p j c", j=g.JT)
            nc.vector.tensor_reduce(winv, cv3, axis=mybir.AxisListType.X,
                                    op=mybir.AluOpType.max)
            eqm = cand_p.tile([64, g.CAND], mybir.dt.int32, tag="eqm")
            eqm3 = eqm[:].rearrange("p (j c) -> p j c", j=g.JT)
            wb3 = winv.unsqueeze(2).broadcast_to([64, g.JT, G8])
            nc.vector.tensor_tensor(eqm3, cv3, wb3,
                                    op=mybir.AluOpType.is_equal)
            selm = cand_p.tile([64, g.CAND], F32, tag="selm")
            selm3 = selm[:].rearrange("p (j c) -> p j c", j=g.JT)
            bb3 = bigG8.unsqueeze(1).broadcast_to([64, g.JT, G8])
            nc.vector.select(selm3, eqm3, ci3, bb3)
            nc.vector.tensor_reduce(wini, selm3, axis=mybir.AxisListType.X,
                                    op=mybir.AluOpType.min)

            # pack (value, index) into one sortable float per (b):
            #   packed = (256 - round(clamp(v)/32)) * 2^15 + y
            # AllReduce(min) then resolves max value, ties -> lowest y.
            # All quantities stay exact in fp32 (integers < 2^24); the host
            # replicates this arithmetic bit-for-bit and absorbs the <=32-unit
            # value quantization inside the DELTA re-check band.
            nc.vector.tensor_scalar(kq, winv, 8000.0, -8000.0,
                                    op0=mybir.AluOpType.min,
                                    op1=mybir.AluOpType.max)
            nc.vector.tensor_scalar(kq, kq, 0.03125, 12582912.0,
                                    op0=mybir.AluOpType.mult,
                                    op1=mybir.AluOpType.add)
            nc.vector.tensor_scalar(kq, kq, 12582912.0, None,
                                    op0=mybir.AluOpType.subtract)
            nc.vector.tensor_scalar(pkt, kq, -32768.0, 8388608.0,
                                    op0=mybir.AluOpType.mult,
                                    op1=mybir.AluOpType.add)
            nc.vector.tensor_add(pkt, pkt, wini)

            nc.sync.dma_start(
                out=ccin[:].rearrange("(p j) -> p j", p=64), in_=pkt)
            nc.gpsimd.collective_compute(
                "AllReduce", mybir.AluOpType.min,
                replica_groups=[list(range(g.NC))],
                ins=[ccin[:].opt()], outs=[ccout[:].opt()])

            # candidate dumps for the host-side margin check (off the
            # critical path)
            nc.sync.dma_start(out=candv_d.ap(), in_=candv_sb[:])
            nc.sync.dma_start(out=candi_d.ap(), in_=candi_sb[:])

            # unpack the global winner index and gather the pre-normalized
            # bf16 y2z rows straight to the output
            prb = post_p.tile([64, g.JT], F32, tag="prb")
            nc.sync.dma_start(out=prb[:],
                              in_=ccout[:].rearrange("(p j) -> p j", p=64))
            pru = post_p.tile([64, g.JT], U32, tag="pru")
            nc.vector.tensor_copy(pru[:], prb[:])
            wiu = post_p.tile([64, g.JT], U32, tag="wiu")
            nc.vector.tensor_scalar(wiu[:], pru[:], 32767, None,
                                    op0=mybir.AluOpType.bitwise_and)

            grows = [post_p.tile([64, g.ZPB], BF16, name=f"grow{j}",
                                 tag=f"grow{j}")
                     for j in range(g.JT)]
            for j in range(g.JT):
                nc.gpsimd.indirect_dma_start(
                    out=grows[j][:], out_offset=None,
                    in_=w2t_d.ap(),
                    in_offset=bass.IndirectOffsetOnAxis(
                        ap=wiu[:, j:j + 1], axis=0))
                nc.sync.dma_start(
                    out=out_d.ap()[j * 64:(j + 1) * 64, :],
                    in_=grows[j][:, 0:g.Z])

    _dedup_ldweights(nc.m)
    nc.compile()
    return nc


# --------------------------------------------------------------------------
# host side
# --------------------------------------------------------------------------

def prep_inputs(g: Geom, x, x2y_w, y2z_w, y_age):
    """Shard + lay out the full inputs for the 8 cores."""
    fp8 = ml_dtypes.float8_e4m3
    bf16 = ml_dtypes.bfloat16
    KP = g.KP
    G8 = g.G * 8

    ylc, gperm = _compaction(g.NC, g.GW, y_age, g.Y)
    assert ylc == g.YLC, (ylc, g.YLC)
    valid = gperm >= 0

    xf = np.ascontiguousarray(x.reshape(g.B, g.X))
    # xt image: xt[p, kp*2B + i*B + b] = x[b, (kp*2+i)*128 + p]
    xt = np.ascontiguousarray(
        xf.astype(fp8).T.reshape(KP, 2, P, g.B).transpose(2, 0, 1, 3)
        .reshape(P, KP * 2 * g.B))

    # fold (2^10 * mask / ||w_y||) into the weights, then fp8-quantize
    wn = np.linalg.norm(x2y_w, axis=1)
    scale = (WSCALE / np.maximum(wn, 1e-12)) * (y_age[0] >= 1)
    wq = (x2y_w * scale[:, None].astype(np.float32)).astype(fp8)   # [Y, X]
    # compacted global weight matrix [YT, X] (pads = zero rows)
    wqc = np.zeros((g.YT, g.X), fp8)
    wqc[valid] = wq[gperm[valid]]

    # pre-normalized bf16 output table in compact index space:
    # w2t[yc, z] = y2z_w[z, gperm[yc]]/||y2z_w[z,:]||
    n2 = np.linalg.norm(y2z_w.astype(np.float64), axis=1)
    w2tn = (y2z_w / n2[:, None]).T.astype(bf16)        # [Y, Z]
    w2t = np.zeros((g.YT, g.ZPB), bf16)
    w2t[valid, :g.Z] = w2tn[gperm[valid]]

    in_maps = []
    for c in range(g.NC):
        wslc = wqc[c * g.YLC:(c + 1) * g.YLC, :]          # [YLC, X] fp8
        # wt group chunks: [G, P, KP*2*GW], chunk gi is the SBUF image of
        # one 256-y group (contiguous per partition):
        #   wt[gi, p, kp*2GW + i*GW + n] = w[gi*GW + n, (kp*2+i)*128 + p]
        wt = np.ascontiguousarray(
            wslc.T.reshape(KP, 2, P, g.G, g.GW)
            .transpose(3, 2, 0, 1, 4).reshape(g.G, P, KP * 2 * g.GW))
        cols = np.arange(g.CAND)
        base_row = (c * g.YLC + g.GW * ((cols % G8) // 8)).astype(np.float32)
        base = np.broadcast_to(base_row, (64, g.CAND)).copy()
        in_maps.append({"xt": xt, "wt": wt, "base": base, "w2t": w2t})
    return in_maps


def postprocess(g: Geom, results, x, x2y_w, y2z_w, y_age):
    """Margin check + fp64 rescore of close rows; patch flipped winners."""
    out = np.array(results[0]["out"], dtype=np.float32, copy=True)
    G8 = g.G * 8
    # candidate values/indices -> [B, NC * G8]; b = j*64 + p
    V = np.empty((g.B, g.NC * G8), np.float32)
    I = np.empty((g.B, g.NC * G8), np.float32)
    for c in range(g.NC):
        cv = np.asarray(results[c]["candv"])   # [64, CAND]
        ci = np.asarray(results[c]["candi"])
        for j in range(g.JT):
            V[j * 64:(j + 1) * 64, c * G8:(c + 1) * G8] = \
                cv[:, j * G8:(j + 1) * G8]
            I[j * 64:(j + 1) * 64, c * G8:(c + 1) * G8] = \
                ci[:, j * G8:(j + 1) * G8]

    xf = x.reshape(g.B, g.X).astype(np.float64)
    xn = np.linalg.norm(xf, axis=1)
    mask = (y_age[0] >= 1)
    n2 = np.linalg.norm(y2z_w.astype(np.float64), axis=1)
    _, gperm = _compaction(g.NC, g.GW, y_age, g.Y)

    # replicate the device's packed AllReduce(min) winner resolution
    # bit-for-bit (fp32 clamp/scale/round, 2^15 packing) so the patch logic
    # compares against the row the device actually gathered
    Vc = V.reshape(g.B, g.NC, G8)
    Ic = I.reshape(g.B, g.NC, G8)
    vmax_c = Vc.max(axis=2)                                   # [B, NC] f32
    imin_c = np.where(Vc == vmax_c[:, :, None], Ic, np.inf).min(axis=2)
    kf = np.rint(np.clip(vmax_c, np.float32(-8000.0), np.float32(8000.0))
                 * np.float32(0.03125))
    packed = (256.0 - kf.astype(np.float64)) * 32768.0 \
        + imin_c.astype(np.float64)
    dev_rows = (packed.min(axis=1).astype(np.int64)) & 32767   # [B] compact
    dev_orig = gperm[dev_rows]            # original y of the gathered row
    I_orig = gperm[I.astype(np.int64)]    # candidate orig y (-1 for pads)

    def exact_c(b, ys):
        ys = np.asarray(ys, dtype=np.int64)
        W = x2y_w[ys, :].astype(np.float64)
        c = (W @ xf[b]) / np.linalg.norm(W, axis=1) / xn[b]
        return np.where(mask[ys], c, 0.0)

    n_flagged = n_patched = 0
    full_rows = []
    for b in range(g.B):
        vb, ib = V[b], I[b]
        vmax = vb.max()
        band = 2.0 * DELTA * xn[b] * WSCALE
        in_band = vb >= vmax - band
        if int(in_band.sum()) <= 1:
            continue
        n_flagged += 1
        # guard: if any group's 8th (weakest reported) candidate is still in
        # band, candidates may be missing -> full exact rescore of the row
        tails = vb.reshape(-1, 8)[:, 7]
        if np.any(tails >= vmax - band):
            full_rows.append(b)
        else:
            dev_w = int(dev_orig[b])
            ys = np.unique(I_orig[b][in_band])
            ys = ys[ys >= 0]              # pads can't be true winners
            ce = exact_c(b, ys)
            cbest = ce.max()
            w_true = int(ys[ce == cbest].min())
            if w_true != dev_w:
                n_patched += 1
                out[b, :] = (y2z_w[:, w_true].astype(np.float64)
                             / n2).astype(np.float32)
    if full_rows:
        W = x2y_w.astype(np.float64)
        wnorm = np.linalg.norm(W, axis=1)
        call = (xf[full_rows] @ W.T) / wnorm[None, :] \
            / xn[full_rows][:, None]
        call = np.where(mask[None, :], call, 0.0)
        for r, b in enumerate(full_rows):
            dev_w = int(dev_orig[b])
            cbest = call[r].max()
            w_true = int(np.nonzero(call[r] == cbest)[0].min())
            if w_true != dev_w:
                n_patched += 1
                out[b, :] = (y2z_w[:, w_true].astype(np.float64)
                             / n2).astype(np.float32)
    postprocess.stats = {"flagged": n_flagged, "patched": n_patched,
                         "full_rescore": len(full_rows)}
    return out


_BUILT = {}


def _get_nc(g: Geom):
    if g not in _BUILT:
        _BUILT[g] = build_nc(g)
    return _BUILT[g]


def kernel(**inputs) -> np.ndarray:
    global LAST_RESULTS
    x = np.asarray(inputs["x"], dtype=np.float32)
    x2y_w = np.asarray(inputs["x2y_w"], dtype=np.float32)
    y2z_w = np.asarray(inputs["y2z_w"], dtype=np.float32)
    y_age = np.asarray(inputs["y_age"])
    ylc, _ = _compaction(FULL.NC, FULL.GW, y_age, FULL.Y)
    g = Geom(YLC=ylc)

    nc = _get_nc(g)
    in_maps = prep_inputs(g, x, x2y_w, y2z_w, y_age)
    res = run_bass_kernel_spmd(nc, in_maps, list(range(g.NC)),
                               trace=TRACE, **TRACE_KWARGS)
    LAST_RESULTS = res
    return postprocess(g, res.results, x, x2y_w, y2z_w, y_age)
